# revision 1
# baseline (speedup 1.0000x reference)
"""GQA attention kernel for Trainium2 (8 NeuronCores).

Problem: B=2, S=2048, D=2048, H=16 heads of DH=128, KV=4 kv heads, G=4
query heads per kv head.  Full (dense) attention, fp32 I/O.

Sharding: batch (2) x kv-head (4) = 8 cores, zero redundant FLOPs.
Each core computes, for its (batch b, kv head h):
    Q_g = x_b @ Wq[:, h,g]  (4 query heads), K = x_b @ Wk[:, h],
    V = x_b @ Wv[:, h], O_g = softmax(Q_g K^T / sqrt(DH)) V,
    y_partial = concat_g(O_g) @ Wo[h-rows, :]
Host sums the 4 kv-head partials per batch and adds bo.

On-chip strategy (all matmuls bf16 with fp32 PSUM accumulation):
 - host pre-transposes x (xT: [D, S]) and pre-casts weights to bf16
 - QT/KT computed head-transposed ([dh, s]) with W stationary, xT moving
 - S^T tiles ([k, q]) computed directly (KT-slice stationary, QT moving)
   so exp(S^T) lands in SBUF already transposed for the AV matmul:
   no P-transpose pass, no max-subtraction (scores are O(few), exp safe)
 - rowsum via ones-vector matmul accumulated alongside AV
 - softmax normalization folded into the PSUM->SBUF copy of O^T
   (multiply by DMA-broadcast 1/rowsum row)
 - out-proj: O^T stationary, Wo moving -> y natural, DMA'd straight
   from PSUM to DRAM.
"""

import sys

if "/opt/trn_rl_repo" not in sys.path:
    sys.path.insert(0, "/opt/trn_rl_repo")

import numpy as np
import ml_dtypes
from contextlib import ExitStack

B, S, D = 2, 2048, 2048
H, DH, GRP = 16, 128, 4
KV = H // GRP            # 4 kv heads
EH = GRP * DH            # 512 = query-head columns per kv head
SCALE = float(1.0 / np.sqrt(np.float32(DH)))
P = 128                  # partitions
NB = 512                 # matmul moving-dim block (one PSUM bank fp32)


def _emit(ctx, tc, aps, s=S, d=D, debug_taps=None):
    """Emit the per-core program. s, d parameterized for small-shape sim tests."""
    import concourse.bass as bass
    from concourse import mybir

    nc = tc.nc
    bf16 = mybir.dt.bfloat16
    f32 = mybir.dt.float32
    Exp = mybir.ActivationFunctionType.Exp
    Identity = mybir.ActivationFunctionType.Identity

    xt, wq, wk, wv, wo, bq, bk, bv, y = (
        aps["xt"], aps["wq"], aps["wk"], aps["wv"], aps["wo"],
        aps["bq"], aps["bk"], aps["bv"], aps["y"],
    )
    nt = s // P           # number of 128-tiles along s
    nd = d // P           # number of 128-tiles along d (contraction)
    nsb = s // NB         # number of 512-blocks along s
    ndb = d // NB         # number of 512-blocks along d (out columns)

    persist = ctx.enter_context(tc.tile_pool(name="persist", bufs=1))
    psum = ctx.enter_context(tc.tile_pool(name="psum", bufs=2, space="PSUM"))
    ptpool = ctx.enter_context(tc.tile_pool(name="ptp", bufs=2))
    rpool = ctx.enter_context(tc.tile_pool(name="rp", bufs=2))
    projpool = tc.tile_pool(name="projp", bufs=1)
    projp = projpool.__enter__()

    xt_sb = projp.tile([P, nd, s], bf16)
    wq_sb = projp.tile([P, nd, EH], bf16)
    wk_sb = projp.tile([P, nd, DH], bf16)
    wv_sb = projp.tile([P, nd, DH], bf16)
    wo_sb = persist.tile([P, GRP, d], bf16)
    qt_sb = persist.tile([P, GRP, s], bf16)
    kt_sb = persist.tile([P, s], bf16)
    v_sb = persist.tile([P, nt, DH], bf16)
    ot_sb = persist.tile([P, GRP, s], bf16)
    bq_sb = persist.tile([P, GRP], f32)
    bk_sb = persist.tile([P, 1], f32)
    bvb_sb = persist.tile([P, DH], f32)
    ones_sb = persist.tile([P, 1], bf16)

    nc.vector.memset(ones_sb, 1.0)

    # ---- loads ----
    xt_r = xt.rearrange("(t p) s -> p t s", p=P)
    wq_r = wq.rearrange("(t p) e -> p t e", p=P)
    wk_r = wk.rearrange("(t p) e -> p t e", p=P)
    wv_r = wv.rearrange("(t p) e -> p t e", p=P)
    wo_r = wo.rearrange("(g p) d -> p g d", p=P)
    for t in range(nd):
        nc.sync.dma_start(out=xt_sb[:, t, :], in_=xt_r[:, t, :])
        nc.sync.dma_start(out=wq_sb[:, t, :], in_=wq_r[:, t, :])
        nc.sync.dma_start(out=wk_sb[:, t, :], in_=wk_r[:, t, :])
        nc.sync.dma_start(out=wv_sb[:, t, :], in_=wv_r[:, t, :])
    for g in range(GRP):
        nc.sync.dma_start(out=wo_sb[:, g, :], in_=wo_r[:, g, :])
    nc.sync.dma_start(out=bq_sb, in_=bq.rearrange("(g p) -> p g", p=P))
    nc.sync.dma_start(out=bk_sb, in_=bk.rearrange("(p o) -> p o", o=1))
    # bv broadcast across partitions (varies along free dim of V)
    bv_bcast = bass.AP(tensor=bv.tensor, offset=bv.offset,
                       ap=[[0, P]] + list(bv.ap))
    nc.sync.dma_start(out=bvb_sb, in_=bv_bcast)

    # ---- projections ----
    # QT_g [dh, s] = (Wq_g)^T x^T, + bq*scale, scaled by 1/sqrt(DH)
    for g in range(GRP):
        for sb in range(nsb):
            ps = psum.tile([P, NB], f32, tag="mm")
            for t in range(nd):
                nc.tensor.matmul(
                    ps,
                    lhsT=wq_sb[:, t, g * DH:(g + 1) * DH],
                    rhs=xt_sb[:, t, sb * NB:(sb + 1) * NB],
                    start=(t == 0), stop=(t == nd - 1),
                )
            nc.scalar.activation(
                out=qt_sb[:, g, sb * NB:(sb + 1) * NB], in_=ps,
                func=Identity, bias=bq_sb[:, g:g + 1], scale=SCALE,
            )
    # KT [dh, s]
    for sb in range(nsb):
        ps = psum.tile([P, NB], f32, tag="mm")
        for t in range(nd):
            nc.tensor.matmul(
                ps, lhsT=wk_sb[:, t, :], rhs=xt_sb[:, t, sb * NB:(sb + 1) * NB],
                start=(t == 0), stop=(t == nd - 1),
            )
        nc.scalar.activation(
            out=kt_sb[:, sb * NB:(sb + 1) * NB], in_=ps,
            func=Identity, bias=bk_sb[:, 0:1], scale=1.0,
        )
    # V natural [k, dh] (xT stationary)
    for ki in range(nt):
        ps = psum.tile([P, NB], f32, tag="mm")
        for t in range(nd):
            nc.tensor.matmul(
                ps[:, 0:DH], lhsT=xt_sb[:, t, ki * P:(ki + 1) * P],
                rhs=wv_sb[:, t, :],
                start=(t == 0), stop=(t == nd - 1),
            )
        nc.vector.tensor_add(v_sb[:, ki, :], ps[:, 0:DH], bvb_sb)

    projpool.__exit__(None, None, None)

    # ---- attention ----
    # Software-pipelined: block n's S^T/exp interleave with block n-1's
    # AV + rowsum matmuls so PE never stalls waiting for ScalarE's exp.
    blocks = [(g, qb) for g in range(GRP) for qb in range(nsb)]

    def finish_block(prev):
        pg, pqb, ppt, pps_o, pps_r = prev
        pqsl = slice(pqb * NB, (pqb + 1) * NB)
        rrow = rpool.tile([1, NB], f32, tag="rrow")
        nc.vector.reciprocal(rrow, pps_r)
        rb = rpool.tile([P, NB], f32, tag="rb")
        nc.gpsimd.partition_broadcast(rb, rrow[0:1, :])
        nc.vector.tensor_mul(ot_sb[:, pg, pqsl], pps_o, rb)

    prev = None
    for g, qb in blocks:
        qsl = slice(qb * NB, (qb + 1) * NB)
        pt = ptpool.tile([P, nt, NB], bf16, tag="pt")
        ps_o = psum.tile([P, NB], f32, tag="o")
        ps_r = psum.tile([1, NB], f32, tag="r")
        for ki in range(nt):
            ps_s = psum.tile([P, NB], f32, tag="s")
            nc.tensor.matmul(
                ps_s, lhsT=kt_sb[:, ki * P:(ki + 1) * P],
                rhs=qt_sb[:, g, qsl], start=True, stop=True,
            )
            nc.scalar.activation(out=pt[:, ki, :], in_=ps_s, func=Exp)
            if prev is not None:
                _, _, ppt, pps_o, pps_r = prev
                nc.tensor.matmul(
                    pps_o, lhsT=v_sb[:, ki, :], rhs=ppt[:, ki, :],
                    start=(ki == 0), stop=(ki == nt - 1),
                )
                nc.tensor.matmul(
                    pps_r, lhsT=ones_sb[:, 0:1], rhs=ppt[:, ki, :],
                    start=(ki == 0), stop=(ki == nt - 1),
                )
        if prev is not None:
            finish_block(prev)
        prev = (g, qb, pt, ps_o, ps_r)
    # drain last block
    g, qb, pt, ps_o, ps_r = prev
    for ki in range(nt):
        nc.tensor.matmul(
            ps_o, lhsT=v_sb[:, ki, :], rhs=pt[:, ki, :],
            start=(ki == 0), stop=(ki == nt - 1),
        )
        nc.tensor.matmul(
            ps_r, lhsT=ones_sb[:, 0:1], rhs=pt[:, ki, :],
            start=(ki == 0), stop=(ki == nt - 1),
        )
    finish_block(prev)

    if debug_taps is not None:
        for name, t in [("qt", qt_sb), ("kt", kt_sb), ("v", v_sb),
                        ("ot", ot_sb), ("pt_last", None)]:
            if name in debug_taps and t is not None:
                nc.sync.dma_start(out=debug_taps[name], in_=t[:])

    # ---- out projection ----
    ypool = ctx.enter_context(tc.tile_pool(name="yp", bufs=2))
    for st in range(nt):
        for db in range(ndb):
            ps_y = psum.tile([P, NB], f32, tag="mm")
            for g in range(GRP):
                nc.tensor.matmul(
                    ps_y, lhsT=ot_sb[:, g, st * P:(st + 1) * P],
                    rhs=wo_sb[:, g, db * NB:(db + 1) * NB],
                    start=(g == 0), stop=(g == GRP - 1),
                )
            y_sb = ypool.tile([P, NB], f32, tag="y")
            if (st * ndb + db) % 2 == 0:
                nc.scalar.copy(y_sb, ps_y)
            else:
                nc.vector.tensor_copy(y_sb, ps_y)
            nc.sync.dma_start(
                out=y[st * P:(st + 1) * P, db * NB:(db + 1) * NB], in_=y_sb)


def build_program(s=S, d=D, debug=False):
    import concourse.tile as tile
    from concourse import bacc, mybir

    nc = bacc.Bacc("TRN2", target_bir_lowering=False, debug=False)
    bf16 = mybir.dt.bfloat16
    f32 = mybir.dt.float32
    aps = {
        "xt": nc.dram_tensor("xt", [d, s], bf16, kind="ExternalInput").ap(),
        "wq": nc.dram_tensor("wq", [d, EH], bf16, kind="ExternalInput").ap(),
        "wk": nc.dram_tensor("wk", [d, DH], bf16, kind="ExternalInput").ap(),
        "wv": nc.dram_tensor("wv", [d, DH], bf16, kind="ExternalInput").ap(),
        "wo": nc.dram_tensor("wo", [EH, d], bf16, kind="ExternalInput").ap(),
        "bq": nc.dram_tensor("bq", [EH], f32, kind="ExternalInput").ap(),
        "bk": nc.dram_tensor("bk", [DH], f32, kind="ExternalInput").ap(),
        "bv": nc.dram_tensor("bv", [DH], f32, kind="ExternalInput").ap(),
        "y": nc.dram_tensor("y", [s, d], f32, kind="ExternalOutput").ap(),
    }
    debug_taps = None
    if debug:
        nt = s // P
        debug_taps = {
            "qt": nc.dram_tensor("dbg_qt", [P, GRP, s], bf16, kind="ExternalOutput").ap(),
            "kt": nc.dram_tensor("dbg_kt", [P, s], bf16, kind="ExternalOutput").ap(),
            "v": nc.dram_tensor("dbg_v", [P, nt, DH], bf16, kind="ExternalOutput").ap(),
            "ot": nc.dram_tensor("dbg_ot", [P, GRP, s], bf16, kind="ExternalOutput").ap(),
        }
    with tile.TileContext(nc) as tc:
        with ExitStack() as ctx:
            _emit(ctx, tc, aps, s=s, d=d, debug_taps=debug_taps)
    nc.compile()
    return nc


def make_in_maps(x, Wq, bq, Wk, bk, Wv, bv, Wo, bo):
    bf = ml_dtypes.bfloat16
    in_maps = []
    for b in range(B):
        xt_b = x[b].T.astype(bf)  # [D, S] contiguous
        for h in range(KV):
            in_maps.append({
                "xt": xt_b,
                "wq": Wq[:, h * EH:(h + 1) * EH].astype(bf),
                "wk": Wk[:, h * DH:(h + 1) * DH].astype(bf),
                "wv": Wv[:, h * DH:(h + 1) * DH].astype(bf),
                "wo": np.ascontiguousarray(Wo[h * EH:(h + 1) * EH, :]).astype(bf),
                "bq": (bq[h * EH:(h + 1) * EH] * SCALE).astype(np.float32),
                "bk": np.ascontiguousarray(bk[h * DH:(h + 1) * DH]).astype(np.float32),
                "bv": np.ascontiguousarray(bv[h * DH:(h + 1) * DH]).astype(np.float32),
            })
    return in_maps


_PROG = None


def _get_program():
    global _PROG
    if _PROG is None:
        _PROG = build_program()
    return _PROG


def run_cores(in_maps, trace=False, **kw):
    from concourse.bass_utils import run_bass_kernel_spmd
    nc = _get_program()
    return run_bass_kernel_spmd(nc, in_maps, list(range(8)), trace=trace, **kw)


def kernel(**inputs):
    x = np.asarray(inputs["x"], dtype=np.float32)
    Wq = np.asarray(inputs["Wq"], dtype=np.float32)
    bq = np.asarray(inputs["bq"], dtype=np.float32)
    Wk = np.asarray(inputs["Wk"], dtype=np.float32)
    bk = np.asarray(inputs["bk"], dtype=np.float32)
    Wv = np.asarray(inputs["Wv"], dtype=np.float32)
    bv = np.asarray(inputs["bv"], dtype=np.float32)
    Wo = np.asarray(inputs["Wo"], dtype=np.float32)
    bo = np.asarray(inputs["bo"], dtype=np.float32)

    in_maps = make_in_maps(x, Wq, bq, Wk, bk, Wv, bv, Wo, bo)
    res = run_cores(in_maps)
    out = np.empty((B, S, D), dtype=np.float32)
    for b in range(B):
        acc = res.results[b * KV]["y"].astype(np.float32)
        for h in range(1, KV):
            acc = acc + res.results[b * KV + h]["y"]
        out[b] = acc + bo[None, :]
    return out



# revision 14
# speedup vs baseline: 1.4910x; 1.4910x over previous
"""GQA attention kernel for Trainium2 (8 NeuronCores).

Problem: B=2, S=2048, D=2048, H=16 heads of DH=128, KV=4 kv heads, G=4
query heads per kv head.  Full (dense) attention, fp32 I/O.

Sharding: batch (2) x kv-head (4) = 8 cores, zero redundant FLOPs.
Each core computes, for its (batch b, kv head h):
    Q_g = x_b @ Wq[:, h,g]  (4 query heads), K = x_b @ Wk[:, h],
    V = x_b @ Wv[:, h], O_g = softmax(Q_g K^T / sqrt(DH)) V,
    y_partial = concat_g(O_g) @ Wo[h-rows, :]
Host sums the 4 kv-head partials per batch and adds bo.

On-chip strategy:
 - Projections and out-proj run as residual-fp8 DoubleRow matmuls:
   each operand is split (on host for x/W, on chip for O) into
   e4m3 hi + e4m3 lo residual; products hi.hi + hi.lo + lo.hi are kept
   (lo.lo dropped).  3 DoubleRow matmuls per 256-deep contraction pair
   = 1.5 PE cycles/row vs bf16's 2.0, at better-than-bf16 accuracy.
 - Scores S^T tiles ([k, q], lhsT=KT slice, rhs=QT block) and AV
   (lhsT=V tile, rhs=exp tile) in bf16.  1/sqrt(DH) is applied inside
   the exp activation (scale operand), keeping qt/kt at unit scale.
 - exp of the 16 score k-tiles per (g, q-block): 12 tiles on the Act
   engine (native Exp, PSUM pair reads [128,1024]), 4 tiles as
   Schraudolph bit-trick exponentials on DVE/Pool (tensor_scalar
   fp32->int16 of s*A+B, bitcast to bf16), spreading exp across three
   engines so the PE stays the bottleneck.
 - rowsum via DVE pairwise tree-add of exp tiles + one [128,1]-ones
   matmul; reciprocal on DVE; 1/r broadcast on Pool; normalization
   multiply on DVE produces O*16/r fp32, split into e4m3 hi/lo for the
   residual out-proj (Act copy + Pool subtract).
 - y written bf16 (PSUM * 1/1024 scale), host sums partials in fp32.
"""

import sys

if "/opt/trn_rl_repo" not in sys.path:
    sys.path.insert(0, "/opt/trn_rl_repo")

import numpy as np
import ml_dtypes
from contextlib import ExitStack

B, S, D = 2, 2048, 2048
H, DH, GRP = 16, 128, 4
KV = H // GRP            # 4 kv heads
EH = GRP * DH            # 512 = query-head columns per kv head
SCALE = float(1.0 / np.sqrt(np.float32(DH)))
P = 128                  # partitions
NB = 512                 # matmul moving-dim block (one PSUM bank fp32)
WSC = 64.0               # weight fp8 pre-scale
OSC = 16.0               # ot fp8 pre-scale

# Schraudolph exp-approx constants (bf16 bit domain), folding in SCALE
SCH_A = float(128.0 * SCALE / np.log(2.0))
SCH_B = float((127.0 - 0.0579) * 128.0)


def _emit(ctx, tc, aps, s=S, d=D):
    import concourse.bass as bass
    from concourse import mybir

    nc = tc.nc
    bf16 = mybir.dt.bfloat16
    f32 = mybir.dt.float32
    e4 = mybir.dt.float8e4
    i16 = mybir.dt.int16
    DR = mybir.MatmulPerfMode.DoubleRow
    Exp = mybir.ActivationFunctionType.Exp
    Identity = mybir.ActivationFunctionType.Identity

    nt = s // P           # 128-tiles along s
    nd = d // P           # 128-tiles along d (contraction)
    npr = nd // 2         # 256-pairs along d
    nsb = s // NB         # 512-blocks along s
    ndb = d // NB         # 512-blocks along d (out columns)

    persist = ctx.enter_context(tc.tile_pool(name="persist", bufs=1))

    # ---- persistent tiles ----
    wohi_sb = persist.tile([P, GRP, d], e4)
    wolo_sb = persist.tile([P, GRP, d], e4)
    qt_sb = persist.tile([P, GRP, s], bf16)
    kt_sb = persist.tile([P, s], bf16)
    v_sb = persist.tile([P, nt, DH], bf16)
    ot8hi = persist.tile([P, GRP, s], e4)
    ot8lo = persist.tile([P, GRP, s], e4)
    bq_sb = persist.tile([P, GRP], f32)
    bk_sb = persist.tile([P, 1], f32)
    bvb_sb = persist.tile([P, DH], f32)
    zbias = persist.tile([P, 1], f32)
    ones16 = persist.tile([P, 1], bf16)

    nc.vector.memset(ones16, 1.0 / OSC)
    nc.vector.memset(zbias, 0.0)

    # ================= phase P: projections =================
    projpool = tc.tile_pool(name="projp", bufs=1)
    projp = projpool.__enter__()
    psP = tc.tile_pool(name="psP", bufs=2, space="PSUM")
    psPp = psP.__enter__()

    xhi_sb = projp.tile([P, nd, s], e4)
    xlo_sb = projp.tile([P, nd, s], e4)
    wqhi_sb = projp.tile([P, nd, EH], e4)
    wqlo_sb = projp.tile([P, nd, EH], e4)
    wkhi_sb = projp.tile([P, nd, DH], e4)
    wklo_sb = projp.tile([P, nd, DH], e4)
    wvhi_sb = projp.tile([P, nd, DH], e4)
    wvlo_sb = projp.tile([P, nd, DH], e4)

    # loads: weights + biases first (gate the first Q/K/V blocks), then x
    # hi/lo in s-column chunks (sb-major) so proj blocks unblock per sb,
    # wo last (only needed in phase O).
    nc.sync.dma_start(out=wqhi_sb, in_=aps["wqhi"].rearrange("(t p) e -> p t e", p=P))
    nc.sync.dma_start(out=wqlo_sb, in_=aps["wqlo"].rearrange("(t p) e -> p t e", p=P))
    nc.sync.dma_start(out=bq_sb, in_=aps["bq"].rearrange("(g p) -> p g", p=P))
    nc.sync.dma_start(out=bk_sb, in_=aps["bk"].rearrange("(p o) -> p o", o=1))
    bv = aps["bv"]
    bv_bcast = bass.AP(tensor=bv.tensor, offset=bv.offset,
                       ap=[[0, P]] + list(bv.ap))
    nc.sync.dma_start(out=bvb_sb, in_=bv_bcast)
    nc.sync.dma_start(out=wkhi_sb, in_=aps["wkhi"].rearrange("(t p) e -> p t e", p=P))
    nc.sync.dma_start(out=wklo_sb, in_=aps["wklo"].rearrange("(t p) e -> p t e", p=P))
    nc.sync.dma_start(out=wvhi_sb, in_=aps["wvhi"].rearrange("(t p) e -> p t e", p=P))
    nc.sync.dma_start(out=wvlo_sb, in_=aps["wvlo"].rearrange("(t p) e -> p t e", p=P))
    xhi_r = aps["xhi"].rearrange("(t p) s -> p t s", p=P)
    xlo_r = aps["xlo"].rearrange("(t p) s -> p t s", p=P)
    for sb in range(nsb):
        ssl = slice(sb * NB, (sb + 1) * NB)
        for t in range(nd):
            nc.sync.dma_start(out=xhi_sb[:, t, ssl], in_=xhi_r[:, t, ssl])
            nc.sync.dma_start(out=xlo_sb[:, t, ssl], in_=xlo_r[:, t, ssl])
    nc.sync.dma_start(out=wohi_sb, in_=aps["wohi"].rearrange("(g p) d -> p g d", p=P))
    nc.sync.dma_start(out=wolo_sb, in_=aps["wolo"].rearrange("(g p) d -> p g d", p=P))

    def res_mm(ps, w_hi, w_lo, x_hi, x_lo, pr, first, last):
        """3 DoubleRow matmuls for one 256-deep pair: hi.hi + hi.lo + lo.hi.
        w_* are lhsT [P, 2, m] APs, x_* are rhs [P, 2, n] APs."""
        nc.tensor.matmul(ps, lhsT=w_hi, rhs=x_hi, start=first, stop=False,
                         perf_mode=DR)
        nc.tensor.matmul(ps, lhsT=w_lo, rhs=x_hi, start=False, stop=False,
                         perf_mode=DR)
        nc.tensor.matmul(ps, lhsT=w_hi, rhs=x_lo, start=False, stop=last,
                         perf_mode=DR)

    # sb-major so each s-block's Q/K/V runs as soon as its x columns land
    for sb in range(nsb):
        ssl = slice(sb * NB, (sb + 1) * NB)
        # Q blocks: QT[dh, s] per g
        for g in range(GRP):
            gsl = slice(g * DH, (g + 1) * DH)
            ps = psPp.tile([P, NB], f32, tag="mm")
            for pr in range(npr):
                jsl = slice(2 * pr, 2 * pr + 2)
                res_mm(ps, wqhi_sb[:, jsl, gsl], wqlo_sb[:, jsl, gsl],
                       xhi_sb[:, jsl, ssl], xlo_sb[:, jsl, ssl],
                       pr, pr == 0, pr == npr - 1)
            nc.scalar.activation(out=qt_sb[:, g, ssl], in_=ps, func=Identity,
                                 bias=bq_sb[:, g:g + 1], scale=1.0 / WSC)
        # K block: KT[dh, s]
        ps = psPp.tile([P, NB], f32, tag="mm")
        for pr in range(npr):
            jsl = slice(2 * pr, 2 * pr + 2)
            res_mm(ps, wkhi_sb[:, jsl, :], wklo_sb[:, jsl, :],
                   xhi_sb[:, jsl, ssl], xlo_sb[:, jsl, ssl],
                   pr, pr == 0, pr == npr - 1)
        nc.scalar.activation(out=kt_sb[:, ssl], in_=ps, func=Identity,
                             bias=bk_sb[:, 0:1], scale=1.0 / WSC)
        # V tiles: V[k, dh] natural (x stationary, wv moving)
        for st in range(4 * sb, 4 * sb + 4):
            tsl = slice(st * P, (st + 1) * P)
            ps = psPp.tile([P, NB], f32, tag="mm")
            for pr in range(npr):
                jsl = slice(2 * pr, 2 * pr + 2)
                res_mm(ps[:, 0:DH], xhi_sb[:, jsl, tsl], xlo_sb[:, jsl, tsl],
                       wvhi_sb[:, jsl, :], wvlo_sb[:, jsl, :],
                       pr, pr == 0, pr == npr - 1)
            nc.vector.scalar_tensor_tensor(
                out=v_sb[:, st, :], in0=ps[:, 0:DH], scalar=1.0 / WSC,
                in1=bvb_sb, op0=mybir.AluOpType.mult, op1=mybir.AluOpType.add)

    projpool.__exit__(None, None, None)
    psP.__exit__(None, None, None)

    # ================= phase A: attention =================
    psSpool = tc.tile_pool(name="psS", bufs=2, space="PSUM")
    psS = psSpool.__enter__()
    psOApool = tc.tile_pool(name="psOA", bufs=3, space="PSUM")
    psOA = psOApool.__enter__()
    psRpool = tc.tile_pool(name="psR", bufs=1, space="PSUM")
    psR = psRpool.__enter__()
    ptpool = ctx.enter_context(tc.tile_pool(name="ptp", bufs=2))
    trpool = ctx.enter_context(tc.tile_pool(name="trp", bufs=2))
    scpool = ctx.enter_context(tc.tile_pool(name="scp", bufs=2))

    npair = nt // 2
    blocks = [(g, qb) for g in range(GRP) for qb in range(nsb)]

    def emit_tree(tree, tall, acc512):
        """Finish a block's rowsum tree: combine the 4 level-1 pair-sums and
        fold [P,2,NB] -> [P,NB]."""
        nc.vector.tensor_add(tall, tree[0], tree[1])
        nc.vector.tensor_add(tall, tall, tree[2])
        nc.vector.tensor_add(tall, tall, tree[3])
        nc.vector.tensor_add(acc512, tall[:, 0, :], tall[:, 1, :])

    def finish_norm(pg, pqb, pacc, pps_o):
        """Rowsum matmul + normalize + fp8 hi/lo split for a finished block.
        PE: 1 ones-matmul; DVE: recip + mul; Pool: bcast + hi copy + lo sub."""
        pqsl = slice(pqb * NB, (pqb + 1) * NB)
        ps_r = psR.tile([1, NB], f32, tag="r")
        nc.tensor.matmul(ps_r, lhsT=ones16[:, 0:1], rhs=pacc,
                         start=True, stop=True)
        rrow = scpool.tile([1, NB], f32, tag="rrow")
        nc.vector.reciprocal(rrow, ps_r)
        rb = scpool.tile([P, NB], f32, tag="rb")
        nc.gpsimd.partition_broadcast(rb, rrow[0:1, :])
        otf = scpool.tile([P, NB], f32, tag="otf")
        nc.vector.tensor_mul(otf, pps_o, rb)
        nc.gpsimd.tensor_copy(ot8hi[:, pg, pqsl], otf)
        nc.gpsimd.tensor_sub(ot8lo[:, pg, pqsl], otf, ot8hi[:, pg, pqsl])

    def emit_block(g, qb, prev):
        """Emit one (g, q-block): scores+exp for this block interleaved on PE
        with AV of `prev`; rowsum/normalize of `prev` rides along at the end.
        Returns this block's state tuple."""
        qsl = slice(qb * NB, (qb + 1) * NB)
        pt = ptpool.tile([P, nt, NB], bf16, tag="pt")
        ps_o = psOA.tile([P, NB], f32, tag="o")
        tree = [trpool.tile([P, 2, NB], bf16, tag=f"t{i}", name=f"tree{i}")
                for i in range(4)]
        tall = trpool.tile([P, 2, NB], bf16, tag="tall")
        acc512 = trpool.tile([P, NB], bf16, tag="acc")
        if prev is not None:
            (pg, pqb, ppt, pps_o, ptree, ptall, pacc) = prev

        for p in range(npair):
            ps_s = psS.tile([P, 2, NB], f32, tag="s")
            for j in (0, 1):
                ki = 2 * p + j
                nc.tensor.matmul(
                    ps_s[:, j, :], lhsT=kt_sb[:, ki * P:(ki + 1) * P],
                    rhs=qt_sb[:, g, qsl], start=True, stop=True)
            # exp of the pair: Act p0-5 native, DVE p6 Schraudolph, p7 split
            # Act/DVE (GPSIMD cannot read PSUM, so Pool gets no exp work)
            if p < 6:
                nc.scalar.activation(
                    out=pt[:, 2 * p:2 * p + 2, :], in_=ps_s, func=Exp,
                    bias=zbias[:, 0:1], scale=SCALE)
            elif p == 6:
                nc.vector.tensor_scalar(
                    out=pt[:, 12:14, :].bitcast(i16), in0=ps_s,
                    scalar1=SCH_A, scalar2=SCH_B,
                    op0=mybir.AluOpType.mult, op1=mybir.AluOpType.add)
            else:
                nc.scalar.activation(
                    out=pt[:, 14:15, :], in_=ps_s[:, 0, :], func=Exp,
                    bias=zbias[:, 0:1], scale=SCALE)
                nc.vector.tensor_scalar(
                    out=pt[:, 15:16, :].bitcast(i16), in0=ps_s[:, 1, :],
                    scalar1=SCH_A, scalar2=SCH_B,
                    op0=mybir.AluOpType.mult, op1=mybir.AluOpType.add)
            # pairwise rowsum tree level 1 as tiles become ready
            if p % 2 == 1:
                nc.vector.tensor_add(tree[p // 2], pt[:, 2 * p - 2:2 * p, :],
                                     pt[:, 2 * p:2 * p + 2, :])
            # AV of prev interleaves with this block's scores on PE
            if prev is not None:
                nc.tensor.matmul(
                    pps_o, lhsT=v_sb[:, 2 * p, :], rhs=ppt[:, 2 * p, :],
                    start=(p == 0), stop=False)
                nc.tensor.matmul(
                    pps_o, lhsT=v_sb[:, 2 * p + 1, :], rhs=ppt[:, 2 * p + 1, :],
                    start=False, stop=(p == npair - 1))
        # fold THIS block's tree right after its last level-1 add so the DVE
        # has pacc ready well before next block's end-of-stream ones-matmul
        emit_tree(tree, tall, acc512)
        if prev is not None:
            finish_norm(pg, pqb, pacc, pps_o)
        return (g, qb, pt, ps_o, tree, tall, acc512)

    prev = None
    for g, qb in blocks:
        prev = emit_block(g, qb, prev)

    # epilogue: drain last block (AV + rowsum + normalize); its tree was
    # already folded inside emit_block
    (pg, pqb, ppt, pps_o, ptree, ptall, pacc) = prev
    for ki in range(nt):
        nc.tensor.matmul(pps_o, lhsT=v_sb[:, ki, :], rhs=ppt[:, ki, :],
                         start=(ki == 0), stop=(ki == nt - 1))
    finish_norm(pg, pqb, pacc, pps_o)
    psRpool.__exit__(None, None, None)
    psOApool.__exit__(None, None, None)
    psSpool.__exit__(None, None, None)

    # ================= phase O: out-projection =================
    psO = ctx.enter_context(tc.tile_pool(name="psO", bufs=4, space="PSUM"))
    ypool = ctx.enter_context(tc.tile_pool(name="yp", bufs=3))
    y = aps["y"]
    ngp = GRP // 2
    for st in range(nt):
        tsl = slice(st * P, (st + 1) * P)
        y_sb = ypool.tile([P, ndb, NB], bf16, tag="y")
        for db in range(ndb):
            dsl = slice(db * NB, (db + 1) * NB)
            ps_y = psO.tile([P, NB], f32, tag="y")
            for gp in range(ngp):
                jsl = slice(2 * gp, 2 * gp + 2)
                res_mm(ps_y, ot8hi[:, jsl, tsl], ot8lo[:, jsl, tsl],
                       wohi_sb[:, jsl, dsl], wolo_sb[:, jsl, dsl],
                       gp, gp == 0, gp == ngp - 1)
            if db % 2 == 0:
                nc.scalar.mul(y_sb[:, db, :], ps_y, 1.0 / (OSC * WSC))
            else:
                nc.vector.tensor_scalar_mul(y_sb[:, db, :], ps_y, 1.0 / (OSC * WSC))
        nc.sync.dma_start(out=y[tsl, :], in_=y_sb)


def build_program(s=S, d=D):
    import concourse.tile as tile
    from concourse import bacc, mybir

    nc = bacc.Bacc("TRN2", target_bir_lowering=False, debug=False)
    bf16 = mybir.dt.bfloat16
    f32 = mybir.dt.float32
    e4 = mybir.dt.float8e4
    names = {
        "xhi": ([d, s], e4), "xlo": ([d, s], e4),
        "wqhi": ([d, EH], e4), "wqlo": ([d, EH], e4),
        "wkhi": ([d, DH], e4), "wklo": ([d, DH], e4),
        "wvhi": ([d, DH], e4), "wvlo": ([d, DH], e4),
        "wohi": ([EH, d], e4), "wolo": ([EH, d], e4),
        "bq": ([EH], f32), "bk": ([DH], f32), "bv": ([DH], f32),
    }
    aps = {k: nc.dram_tensor(k, sh, dt, kind="ExternalInput").ap()
           for k, (sh, dt) in names.items()}
    aps["y"] = nc.dram_tensor("y", [s, d], bf16, kind="ExternalOutput").ap()
    with tile.TileContext(nc) as tc:
        with ExitStack() as ctx:
            _emit(ctx, tc, aps, s=s, d=d)
    nc.compile()
    return nc


def _res_split(v32, scale):
    """fp8 residual split of v32*scale: hi = e4(v*scale), lo = e4(v*scale-hi)."""
    e4 = ml_dtypes.float8_e4m3
    vs = v32 * np.float32(scale)
    hi = vs.astype(e4)
    lo = (vs - hi.astype(np.float32)).astype(e4)
    return hi, lo


def make_in_maps(x, Wq, bq, Wk, bk, Wv, bv, Wo, bo):
    in_maps = []
    xsplit = []
    for b in range(B):
        xT = np.ascontiguousarray(x[b].T)  # [D, S]
        xsplit.append(_res_split(xT, 1.0))
    for b in range(B):
        xhi, xlo = xsplit[b]
        for h in range(KV):
            wqh, wql = _res_split(np.ascontiguousarray(
                Wq[:, h * EH:(h + 1) * EH]), WSC)
            wkh, wkl = _res_split(np.ascontiguousarray(
                Wk[:, h * DH:(h + 1) * DH]), WSC)
            wvh, wvl = _res_split(np.ascontiguousarray(
                Wv[:, h * DH:(h + 1) * DH]), WSC)
            woh, wol = _res_split(np.ascontiguousarray(
                Wo[h * EH:(h + 1) * EH, :]), WSC)
            in_maps.append({
                "xhi": xhi, "xlo": xlo,
                "wqhi": wqh, "wqlo": wql,
                "wkhi": wkh, "wklo": wkl,
                "wvhi": wvh, "wvlo": wvl,
                "wohi": woh, "wolo": wol,
                "bq": np.ascontiguousarray(bq[h * EH:(h + 1) * EH]).astype(np.float32),
                "bk": np.ascontiguousarray(bk[h * DH:(h + 1) * DH]).astype(np.float32),
                "bv": np.ascontiguousarray(bv[h * DH:(h + 1) * DH]).astype(np.float32),
            })
    return in_maps


_PROG = None


def _get_program():
    global _PROG
    if _PROG is None:
        _PROG = build_program()
    return _PROG


def run_cores(in_maps, trace=False, **kw):
    from concourse.bass_utils import run_bass_kernel_spmd
    nc = _get_program()
    return run_bass_kernel_spmd(nc, in_maps, list(range(8)), trace=trace, **kw)


def kernel(**inputs):
    x = np.asarray(inputs["x"], dtype=np.float32)
    Wq = np.asarray(inputs["Wq"], dtype=np.float32)
    bq = np.asarray(inputs["bq"], dtype=np.float32)
    Wk = np.asarray(inputs["Wk"], dtype=np.float32)
    bk = np.asarray(inputs["bk"], dtype=np.float32)
    Wv = np.asarray(inputs["Wv"], dtype=np.float32)
    bv = np.asarray(inputs["bv"], dtype=np.float32)
    Wo = np.asarray(inputs["Wo"], dtype=np.float32)
    bo = np.asarray(inputs["bo"], dtype=np.float32)

    in_maps = make_in_maps(x, Wq, bq, Wk, bk, Wv, bv, Wo, bo)
    res = run_cores(in_maps)
    out = np.empty((B, S, D), dtype=np.float32)
    for b in range(B):
        acc = res.results[b * KV]["y"].astype(np.float32)
        for h in range(1, KV):
            acc = acc + res.results[b * KV + h]["y"].astype(np.float32)
        out[b] = acc + bo[None, :]
    return out
